# revision 23
# baseline (speedup 1.0000x reference)
"""Causal multi-head self-attention with RoPE on 8 Trainium2 NeuronCores.

Sharding: batch (4) x query-half (2) -> 8 cores, no collectives.
Each core computes full K/V for its batch; query rows split between the two
cores of a batch in a causally-balanced schedule: 4 slots of 256 query rows,
slot sl covering C=4(sl+1) key blocks of 128 (j=0 tiles [1,3,5,7], j=1 tiles
[0,2,4,6]; masked tails make the shared SPMD program serve both halves).

Everything flows in fp16 (PSUM accumulation in f32) in transposed
[feature, seq] layout:
  K^T/Q^T = W^T.T @ X^T            per 128-row head pair
  RoPE    = cos*x + sin*(P@x)      (P = pair-rotation matrix)
  S^T     = Krot^T.T @ Qpad        per-head scores via a full 128-deep
                                   contraction against zero-padded Q (the
                                   other head's partition rows are zero), so
                                   no partition-offset matmuls are needed.
                                   2 heads x 2 key blocks pack into one
                                   2-bank PSUM tile -> single [128,1024] Exp.
  A       = exp-stationary @ [V|1]   ([q, 65] outputs, 65-row matmuls;
                                      region-major accumulation: interleaved
                                      start/stop groups in one PSUM bank drop
                                      contributions on HW)
  y       = (A/denominator).T via PE transpose, then @ Wo^T

Engine split: PE matmuls; Act = Exp + PSUM->SBUF K copies; DVE = rope
multiplies (PSUM direct), masks, spills, normalize; Pool (no PSUM access) =
SBUF-only memsets/scales.  Attention is emitted as 2-key-block pair units
software-pipelined into the projection stream so every engine stays busy.
"""

import os
import sys
import math

if "/opt/trn_rl_repo" not in sys.path:
    sys.path.append("/opt/trn_rl_repo")

import numpy as np

import concourse.bass as bass
import concourse.tile as tile
from concourse import bacc, mybir
from concourse.bass_utils import run_bass_kernel_spmd

B = 4
S = 2048
D = 1024
H = 16
DK = 64
THETA = 10000.0

NEP = H // 2          # head pairs (128-partition groups)
QT = 256              # query tile width
KB = 128              # key block
NSLOT = 4
TILES = [[1, 3, 5, 7], [0, 2, 4, 6]]  # slot -> 256-row q-tile, per half
VW = DK + 1           # V columns per head incl. trailing ones column

F32 = mybir.dt.float32
F16 = mybir.dt.float16

_cache = {}


def _build_program():
    if "nc" in _cache:
        return _cache["nc"]

    nc = bacc.Bacc("TRN2")

    xt_d = nc.dram_tensor("xt", [D, S], F16, kind="ExternalInput")
    xq_d = nc.dram_tensor("xq", [D, NSLOT * QT], F16, kind="ExternalInput")
    wkt_d = nc.dram_tensor("wkt", [D, D], F16, kind="ExternalInput")
    wqt_d = nc.dram_tensor("wqt", [D, D], F16, kind="ExternalInput")
    wvt_d = nc.dram_tensor("wvt", [D, D], F16, kind="ExternalInput")
    wot_d = nc.dram_tensor("wot", [D, D], F16, kind="ExternalInput")
    cosk_d = nc.dram_tensor("cosk", [128, S], F16, kind="ExternalInput")
    sink_d = nc.dram_tensor("sink", [128, S], F16, kind="ExternalInput")
    cosq_d = nc.dram_tensor("cosq", [128, NSLOT * QT], F16, kind="ExternalInput")
    sinq_d = nc.dram_tensor("sinq", [128, NSLOT * QT], F16, kind="ExternalInput")
    mask_d = nc.dram_tensor("mask", [128, NSLOT, 4, QT], F16, kind="ExternalInput")
    permt_d = nc.dram_tensor("permt", [128, 128], F16, kind="ExternalInput")
    ident_d = nc.dram_tensor("ident", [128, 128], F16, kind="ExternalInput")
    y_d = nc.dram_tensor("y", [NSLOT * QT, D], F32, kind="ExternalOutput")

    xt_t = xt_d.rearrange("(n p) s -> p n s", p=128)
    xq_t = xq_d.rearrange("(n p) s -> p n s", p=128)
    wkt_t = wkt_d.rearrange("(n p) e -> p n e", p=128)
    wqt_t = wqt_d.rearrange("(n p) e -> p n e", p=128)
    wvt_t = wvt_d.rearrange("(n p) e -> p n e", p=128)
    wot_t = wot_d.rearrange("(n p) e -> p n e", p=128)

    from contextlib import ExitStack

    with tile.TileContext(nc) as tc:
        with ExitStack() as stack:
            pool = lambda *a, **k: stack.enter_context(tc.tile_pool(*a, **k))
            cpool = pool(name="const", bufs=1)
            wkp = pool(name="wk", bufs=1)
            wqp = pool(name="wq", bufs=1)
            wvp = pool(name="wv", bufs=1)
            wop = pool(name="wo", bufs=1)
            ckp = pool(name="ck", bufs=1)
            cqp = pool(name="cq", bufs=1)
            kvp = pool(name="kv", bufs=1)
            qp = pool(name="qr", bufs=1)
            mkp = pool(name="mk", bufs=1)
            xsp = pool(name="xs", bufs=3)
            xqp = pool(name="xq", bufs=2)
            stg = pool(name="stg", bufs=2)
            exp_p = pool(name="ex", bufs=2)
            accsp = pool(name="acs", bufs=1)
            nrmp = pool(name="nrm", bufs=3)
            atp = pool(name="at", bufs=1)
            outs = pool(name="ot", bufs=1)
            pjp = pool(name="pj", bufs=2, space="PSUM")
            pbp = pool(name="pb", bufs=2, space="PSUM")
            pap = pool(name="pa", bufs=2, space="PSUM")

            # ------------- persistent tiles + preload DMAs (in need order) ---
            xs_t = {}   # (st, xh) -> [128, 8, 256]

            def load_xs(st, xh):
                t = xsp.tile([128, 8, QT], F16, tag="xs", name=f"xs{st}_{xh}")
                xs_t[(st, xh)] = t
                nc.sync.dma_start(
                    t[:], xt_t[:, :, st * 512 + xh * QT:st * 512 + (xh + 1) * QT])

            load_xs(0, 0)
            load_xs(0, 1)
            permt = cpool.tile([128, 128], F16)
            nc.sync.dma_start(permt[:], permt_d[:])

            wk = [wkp.tile([128, 8, 128], F16, tag=f"wk{ep}", name=f"wk{ep}")
                  for ep in range(NEP)]
            for ep in range(2):
                nc.scalar.dma_start(wk[ep][:],
                                    wkt_t[:, :, ep * 128:(ep + 1) * 128])
            cosk = ckp.tile([128, S], F16)
            sink = ckp.tile([128, S], F16)
            nc.scalar.dma_start(cosk[:], cosk_d[:])
            nc.scalar.dma_start(sink[:], sink_d[:])
            for ep in range(2, NEP):
                nc.scalar.dma_start(wk[ep][:],
                                    wkt_t[:, :, ep * 128:(ep + 1) * 128])

            xqs_t = {}
            xqs_t[0] = xqp.tile([128, 8, QT], F16, tag="xq", name="xq0")
            nc.sync.dma_start(xqs_t[0][:], xq_t[:, :, 0:QT])

            wvtTt = [wvp.tile([128, 8, 512], F16, name=f"wvtT{i}")
                     for i in range(2)]
            nc.sync.dma_start(wvtTt[0][:], wvt_t[:, :, 0:512])
            nc.sync.dma_start(wvtTt[1][:], wvt_t[:, :, 512:D])

            wqTt = [wqp.tile([128, 8, 512], F16, name=f"wqT{i}")
                    for i in range(2)]
            for i in range(2):
                nc.scalar.dma_start(wqTt[i][:], wqt_t[:, :, i * 512:(i + 1) * 512])

            def wqT(d, lo, hi):
                return wqTt[lo // 512][:, d, lo % 512:(lo % 512) + (hi - lo)]
            cosq = cqp.tile([128, NSLOT * QT], F16)
            sinq = cqp.tile([128, NSLOT * QT], F16)
            nc.scalar.dma_start(cosq[:], cosq_d[:])
            nc.scalar.dma_start(sinq[:], sinq_d[:])

            ident = cpool.tile([128, 128], F16)
            nc.sync.dma_start(ident[:], ident_d[:])
            masks = mkp.tile([128, NSLOT, 4, QT], F16)
            nc.scalar.dma_start(masks[:], mask_d[:])

            load_xs(1, 0)
            load_xs(1, 1)

            # zero-padded qrot: [128, slot, head, q]; head h lives in
            # partition rows h*64..h*64+64, the other rows stay zero.
            qrot = [qp.tile([128, NSLOT, 2, QT], F16, tag=f"qr{ep}",
                            name=f"qr{ep}") for ep in range(NEP)]
            for ep in range(NEP):
                nc.gpsimd.memset(qrot[ep][:], 0.0)
            wotT = wop.tile([128, 8, D], F16, name="wotT")
            nc.gpsimd.dma_start(wotT[:, :, 0:512], wot_t[:, :, 0:512])
            nc.gpsimd.dma_start(wotT[:, :, 512:D], wot_t[:, :, 512:D])

            krot_t = {}   # (st, ep) -> [128, 512]
            vt_t = {}     # (st, half) -> [128, 16, VW]
            acc_s = {}    # (sl, ep) -> [128, 4, VW] f16 spill accumulator
            accp_of = {}  # (sl, ep) -> live PSUM accumulator
            aq_of = {}    # (sl, ep) -> normalized [128, 2, 2, 64] f16
            aT = {}       # ep -> [128, 2, 128] f16
            exctr = [0]   # alternates exp tile tags for deeper pipelining
            e_of = {}     # (sl, ep) -> exp tile of the chunk's first pair

            for sl in range(1, NSLOT):
                for ep in range(NEP):
                    acc_s[(sl, ep)] = accsp.tile(
                        [128, 4, VW], F16, tag=f"as{sl}_{ep}", name=f"as{sl}_{ep}")

            # ---------------- emission units ----------------
            def dma_xs(st, xh):
                def go():
                    load_xs(st, xh)
                return go

            def dma_xq(sl):
                def go():
                    t = xqp.tile([128, 8, QT], F16, tag="xq", name=f"xq{sl}")
                    xqs_t[sl] = t
                    nc.sync.dma_start(t[:], xq_t[:, :, sl * QT:(sl + 1) * QT])
                return go

            def u_K(st, ep):
                def go():
                    pk = pjp.tile([128, 512], F32, tag="pj", name="pk")
                    for xh in range(2):
                        for d in range(8):
                            nc.tensor.matmul(
                                pk[:, xh * QT:(xh + 1) * QT],
                                wk[ep][:, d, :],
                                xs_t[(st, xh)][:, d, :],
                                start=(d == 0), stop=(d == 7))
                    kraw = stg.tile([128, 512], F16, tag="kraw", name="kraw",
                                    bufs=1)
                    nc.scalar.activation(kraw[:], pk[:],
                                         mybir.ActivationFunctionType.Copy)
                    csl = slice(st * 512, (st + 1) * 512)
                    t_c = stg.tile([128, 512], F16, tag="tc", name="tc",
                                   bufs=1)
                    nc.vector.tensor_mul(t_c[:], kraw[:], cosk[:, csl])
                    pp = pjp.tile([128, 512], F32, tag="pj", name="pp")
                    nc.tensor.matmul(pp[:], permt[:], kraw[:],
                                     start=True, stop=True)
                    t_s = stg.tile([128, 512], F16, tag="ts", name="ts",
                                   bufs=1)
                    nc.vector.tensor_mul(t_s[:], pp[:], sink[:, csl])
                    kr = kvp.tile([128, 512], F16, tag=f"kr{ep}",
                                  name=f"kr{ep}_{st}", bufs=2)
                    krot_t[(st, ep)] = kr
                    nc.vector.tensor_add(kr[:], t_c[:], t_s[:])
                return go

            def u_Q(sl, ep):
                def go():
                    xq = xqs_t[sl]
                    pq = pjp.tile([128, 512], F32, tag="pj", name="pq")
                    for d in range(8):
                        nc.tensor.matmul(pq[:, 0:QT],
                                         wqT(d, ep * 128, (ep + 1) * 128),
                                         xq[:, d, :],
                                         start=(d == 0), stop=(d == 7))
                    qraw = stg.tile([128, QT], F16, tag="qraw", name="qraw")
                    nc.vector.tensor_copy(qraw[:], pq[:, 0:QT])
                    csl = slice(sl * QT, (sl + 1) * QT)
                    t_c = stg.tile([128, QT], F16, tag="tcq", name="tcq",
                                   bufs=1)
                    nc.vector.tensor_mul(t_c[:], qraw[:], cosq[:, csl])
                    pp = pjp.tile([128, 512], F32, tag="pj", name="ppq")
                    nc.tensor.matmul(pp[:, 0:QT], permt[:], qraw[:],
                                     start=True, stop=True)
                    t_s = stg.tile([128, QT], F16, tag="tsq", name="tsq",
                                   bufs=1)
                    nc.vector.tensor_mul(t_s[:], pp[:, 0:QT], sinq[:, csl])
                    for h in range(2):
                        nc.vector.tensor_add(
                            qrot[ep][h * 64:(h + 1) * 64, sl, h, :],
                            t_c[h * 64:(h + 1) * 64, :],
                            t_s[h * 64:(h + 1) * 64, :])
                return go

            def u_V(st, half):
                def go():
                    xh, off = half // 2, (half % 2) * KB
                    xs = xs_t[(st, xh)]
                    vtile = kvp.tile([128, H, VW], F16, tag=f"vt{half}",
                                     name=f"vt{half}_{st}", bufs=2)
                    vt_t[(st, half)] = vtile
                    nc.gpsimd.memset(vtile[:, :, DK], 1.0)
                    for et in range(2):
                        pv = pjp.tile([128, 512], F32, tag="pj", name="pv")
                        for d in range(8):
                            nc.tensor.matmul(
                                pv[:], xs[:, d, off:off + KB],
                                wvtTt[et][:, d, :],
                                start=(d == 0), stop=(d == 7))
                        if et == 0:
                            nc.scalar.activation(
                                vtile[:, et * 8:(et + 1) * 8, 0:DK],
                                pv[:].rearrange("p (h w) -> p h w", w=DK),
                                mybir.ActivationFunctionType.Copy)
                        else:
                            nc.vector.tensor_copy(
                                vtile[:, et * 8:(et + 1) * 8, 0:DK],
                                pv[:].rearrange("p (h w) -> p h w", w=DK))
                return go

            def u_pair(sl, cs, ep, pr, sched):
                C = 4 * (sl + 1)

                def go():
                    if pr == 0:
                        accp = pap.tile([128, 4, VW], F32, tag="acc",
                                        name="acc")
                        accp_of[(sl, ep)] = accp
                    else:
                        accp = accp_of[(sl, ep)]
                    psc = pbp.tile([128, 4, QT], F32, tag="pb", name="psc")
                    for i in range(2):
                        kb = 4 * cs + 2 * pr + i
                        kloc = kb - 4 * cs
                        for h in range(2):
                            nc.tensor.matmul(
                                psc[:, i * 2 + h, :],
                                krot_t[(cs, ep)][:, kloc * KB:(kloc + 1) * KB],
                                qrot[ep][:, sl, h, :],
                                start=True, stop=True)
                    tagx = "ex" if exctr[0] % 2 == 0 else "exm"
                    exctr[0] += 1
                    e = exp_p.tile([128, 4, QT], F16, tag=tagx, name="ex")
                    nc.scalar.activation(
                        e[:], psc[:], mybir.ActivationFunctionType.Exp,
                        scale=1.0 / math.sqrt(DK))
                    if 4 * cs + 2 * pr + 1 >= C - 4:
                        for i in range(2):
                            kb = 4 * cs + 2 * pr + i
                            m = kb - (C - 4)
                            for h in range(2):
                                nc.vector.tensor_mul(
                                    e[:, i * 2 + h, :], e[:, i * 2 + h, :],
                                    masks[:, sl, m, :])
                    if pr == 0:
                        e_of[(sl, ep)] = e
                        return

                    e0 = e_of[(sl, ep)]
                    e1 = e

                    def av_and_tail():
                        # Region-major accumulation: interleaved start/stop
                        # groups within one PSUM bank drop contributions on
                        # real HW, so each [q,65] region accumulates its 4
                        # key blocks consecutively.
                        for qh in range(2):
                            for h in range(2):
                                for kk in range(4):
                                    esrc = e0 if kk < 2 else e1
                                    nc.tensor.matmul(
                                        accp[:, qh * 2 + h, :],
                                        esrc[:, (kk % 2) * 2 + h,
                                             qh * 128:(qh + 1) * 128],
                                        vt_t[(cs, kk)][:, 2 * ep + h, :],
                                        start=(kk == 0), stop=(kk == 3))
                        if sl > 0 and cs < sl:
                            if cs == 0:
                                nc.vector.tensor_copy(
                                    acc_s[(sl, ep)][:], accp[:])
                            else:
                                nc.vector.tensor_add(
                                    acc_s[(sl, ep)][:], acc_s[(sl, ep)][:],
                                    accp[:])
                    sched.pend.append([0, av_and_tail])
                return go

            def u_fin_a(sl, ep):
                def go():
                    accp = accp_of[(sl, ep)]
                    if sl == 0:
                        src = accp
                    else:
                        src = nrmp.tile([128, 4, VW], F32, tag="accf",
                                        name="accf")
                        nc.vector.tensor_add(src[:], accp[:],
                                             acc_s[(sl, ep)][:])
                    rcp = nrmp.tile([128, 4], F32, tag="rcp", name="rcp")
                    nc.vector.reciprocal(rcp[:], src[:, :, DK])
                    aq = nrmp.tile([128, 2, 2, DK], F16, tag="aq", name="aq")
                    aq_of[(sl, ep)] = aq
                    for qh in range(2):
                        for h in range(2):
                            i = qh * 2 + h
                            eng = nc.vector if sl == 0 else nc.gpsimd
                            eng.tensor_scalar_mul(
                                aq[:, qh, h, :], src[:, i, 0:DK],
                                rcp[:, i:i + 1])
                return go

            def u_fin_b(sl, ep):
                def go():
                    aq = aq_of[(sl, ep)]
                    ptt = pap.tile([128, 2, 128], F16, tag="acc", name="ptt")
                    for qh in range(2):
                        nc.tensor.transpose(
                            ptt[:, qh, :],
                            aq[:, qh, :, :].rearrange("p a b -> p (a b)"),
                            ident[:])
                    t = atp.tile([128, 2, 128], F16, tag=f"aT{ep}",
                                 name=f"aT{ep}")
                    aT[ep] = t
                    nc.vector.tensor_copy(t[:], ptt[:])
                return go

            def u_out(sl, qs, et):
                def go():
                    po = pjp.tile([128, 512], F32, tag="pj", name="po")
                    for d in range(8):
                        nc.tensor.matmul(
                            po[:], aT[d][:, qs, :],
                            wotT[:, d, et * 512:(et + 1) * 512],
                            start=(d == 0), stop=(d == 7))
                    ot = outs.tile([128, 512], F32, tag="ot", name="ot")
                    if sl == 3:
                        nc.scalar.activation(
                            ot[:], po[:], mybir.ActivationFunctionType.Copy)
                    else:
                        nc.vector.tensor_copy(ot[:], po[:])
                    nc.sync.dma_start(
                        y_d[sl * QT + qs * 128: sl * QT + (qs + 1) * 128,
                            et * 512:(et + 1) * 512], ot[:])
                return go

            # ---------------- schedule ----------------
            class Sched:
                def __init__(self):
                    self.pend = []  # [age, fn]

                def boundary(self, final=False):
                    keep = []
                    for ent in self.pend:
                        if ent[0] >= 2 or final:
                            ent[1]()
                        else:
                            ent[0] += 1
                            keep.append(ent)
                    self.pend = keep

            sched = Sched()

            # Preamble compute: K(0) only; everything else is paced.
            for ep in range(NEP):
                sched.boundary()
                u_K(0, ep)()

            # Projection stream; per-unit thunks, emission-paced.
            P = []
            P += [u_V(0, half) for half in range(4)]          # 0-3
            P += [u_Q(0, ep) for ep in range(NEP)]            # 4-11
            P += [dma_xq(1)]                                  # 12
            P += [u_Q(1, ep) for ep in range(NEP)]            # 13-20
            P += [dma_xq(2), dma_xs(2, 0), dma_xs(2, 1)]      # 21-23
            P += [u_Q(2, ep) for ep in range(NEP)]            # 24-31
            P += [dma_xq(3)]                                  # 32
            P += [u_Q(3, ep) for ep in range(NEP)]            # 33-40
            P += [u_V(1, half) for half in range(4)]          # 41-44
            P += [u_K(1, ep) for ep in range(NEP)]            # 45-52
            P += [dma_xs(3, 0), dma_xs(3, 1)]                 # 53-54
            P += [u_V(2, half) for half in range(4)]          # 55-58
            P += [u_K(2, ep) for ep in range(NEP)]            # 59-66
            P += [u_V(3, half) for half in range(4)]          # 67-70
            P += [u_K(3, ep) for ep in range(NEP)]            # 71-78

            qpos = [5, 14, 25, 34]   # P pos after which Q(sl, ep) is emitted

            def dep_chunk(sl, cs, ep):
                d = qpos[sl] + ep
                if cs == 1:
                    d = max(d, 46 + ep)
                elif cs == 2:
                    d = max(d, 60 + ep)
                elif cs == 3:
                    d = max(d, 72 + ep)
                return d

            def finale_seq(sl):
                seq = []
                for ep in range(NEP):
                    d = dep_chunk(sl, sl, ep)
                    seq.append((d, u_pair(sl, sl, ep, 0, sched)))
                    seq.append((d, u_pair(sl, sl, ep, 1, sched)))
                    if ep >= 1:
                        seq.append((d, u_fin_a(sl, ep - 1)))
                    if ep >= 2:
                        seq.append((d, u_fin_b(sl, ep - 2)))
                dl = dep_chunk(sl, sl, NEP - 1)
                seq.append((dl, u_fin_a(sl, NEP - 1)))
                seq.append((dl, u_fin_b(sl, NEP - 2)))
                seq.append((dl, u_fin_b(sl, NEP - 1)))
                for qs in range(2):
                    for et in range(2):
                        seq.append((dl, u_out(sl, qs, et)))
                return seq

            def merge(a, b):
                """Proportionally interleave two (dep, unit) lists."""
                out = []
                ia = ib = 0
                while ia < len(a) or ib < len(b):
                    if ib >= len(b):
                        out.append(a[ia]); ia += 1
                    elif ia >= len(a):
                        out.append(b[ib]); ib += 1
                    elif ia * len(b) <= ib * len(a):
                        out.append(a[ia]); ia += 1
                    else:
                        out.append(b[ib]); ib += 1
                return out

            R = [None] * 4
            R[0] = merge(finale_seq(0),
                         [(dep_chunk(sl, 0, ep), u_pair(sl, 0, ep, pr, sched))
                          for sl in range(1, 4) for ep in range(NEP)
                          for pr in range(2)])
            R[1] = merge(finale_seq(1),
                         [(dep_chunk(sl, 1, ep), u_pair(sl, 1, ep, pr, sched))
                          for sl in range(2, 4) for ep in range(NEP)
                          for pr in range(2)])
            R[2] = merge(finale_seq(2),
                         [(dep_chunk(3, 2, ep), u_pair(3, 2, ep, pr, sched))
                          for ep in range(NEP) for pr in range(2)])
            R[3] = finale_seq(3)
            A = R[0] + R[1] + R[2] + R[3]

            # piecewise-linear pacing targets (A index, P position): P must
            # reach each round's K/V block by that round's first unit.
            anchors = [(0, 0), (len(R[0]), 55),
                       (len(R[0]) + len(R[1]), 67), (len(A), len(P))]

            def pace(i):
                for (i0, p0), (i1, p1) in zip(anchors, anchors[1:]):
                    if i < i1:
                        return p0 + ((p1 - p0) * (i - i0)) // max(1, i1 - i0)
                return len(P)

            p = 0
            for i, (dep, unit) in enumerate(A):
                target = max(dep, pace(i))
                while p < target and p < len(P):
                    sched.boundary()
                    P[p]()
                    p += 1
                sched.boundary()
                unit()
            while p < len(P):
                sched.boundary()
                P[p]()
                p += 1
            sched.boundary(final=True)
            sched.boundary(final=True)

    nc.compile()
    nc.finalize()
    _cache["nc"] = nc
    return nc


def _rope_tables(pos):
    """cos/sin tables in [128, n] head-pair layout."""
    k = np.arange(DK // 2, dtype=np.float64)
    inv_freq = THETA ** (-2.0 * k / DK)
    ang = inv_freq[:, None] * np.asarray(pos, np.float64)[None, :]  # [32, n]
    cos64 = np.repeat(np.cos(ang), 2, axis=0)
    sin64 = np.repeat(np.sin(ang), 2, axis=0)
    return (np.ascontiguousarray(
                np.concatenate([cos64, cos64], axis=0)).astype(np.float16),
            np.ascontiguousarray(
                np.concatenate([sin64, sin64], axis=0)).astype(np.float16))


def _masks(j):
    """[128, NSLOT, 4, QT] f16 multiplicative causal masks for half j."""
    p = np.arange(KB)[:, None]
    f = np.arange(QT)[None, :]
    triA = (f >= p).astype(np.float32)
    triB = (f >= p + KB).astype(np.float32)
    ones = np.ones((KB, QT), np.float32)
    zeros = np.zeros((KB, QT), np.float32)
    per_slot = [ones, ones, triA, triB] if j == 0 else [triA, triB, zeros, zeros]
    m = np.stack([np.stack(per_slot, axis=0)] * NSLOT, axis=0)  # [slot, 4, p, f]
    return np.ascontiguousarray(m.transpose(2, 0, 1, 3)).astype(np.float16)


def _host_inputs(in_features, token_positions, Wq, Wk, Wv, Wo):
    X = np.asarray(in_features, dtype=np.float32)
    pos = np.asarray(token_positions)
    wqt = np.ascontiguousarray(np.asarray(Wq, np.float32).T).astype(np.float16)
    wkt = np.ascontiguousarray(np.asarray(Wk, np.float32).T).astype(np.float16)
    wvt = np.ascontiguousarray(np.asarray(Wv, np.float32).T).astype(np.float16)
    wot = np.ascontiguousarray(np.asarray(Wo, np.float32).T).astype(np.float16)
    cosk, sink = _rope_tables(pos)

    permt = np.zeros((128, 128), np.float16)
    for i in range(64):
        permt[2 * i + 1, 2 * i] = -1.0
        permt[2 * i, 2 * i + 1] = 1.0
    ident = np.eye(128, dtype=np.float16)

    in_maps = []
    for core in range(8):
        b, j = core // 2, core % 2
        rows = np.concatenate(
            [np.arange(t * QT, (t + 1) * QT) for t in TILES[j]])
        cosq, sinq = _rope_tables(pos[rows])
        in_maps.append({
            "xt": np.ascontiguousarray(X[b].T).astype(np.float16),
            "xq": np.ascontiguousarray(X[b][rows].T).astype(np.float16),
            "wkt": wkt, "wqt": wqt, "wvt": wvt, "wot": wot,
            "cosk": cosk, "sink": sink, "cosq": cosq, "sinq": sinq,
            "mask": _masks(j), "permt": permt, "ident": ident,
        })
    return in_maps


def kernel(in_features, token_positions, Wq, Wk, Wv, Wo):
    nc = _build_program()
    in_maps = _host_inputs(in_features, token_positions, Wq, Wk, Wv, Wo)

    trace = bool(int(os.environ.get("KERNEL_TRACE", "0")))
    res = run_bass_kernel_spmd(nc, in_maps, core_ids=list(range(8)), trace=trace)
    kernel.last_result = res

    out = np.empty((B, S, D), np.float32)
    for core in range(8):
        b, j = core // 2, core % 2
        y = res.results[core]["y"]
        for sl, t in enumerate(TILES[j]):
            out[b, t * QT:(t + 1) * QT, :] = y[sl * QT:(sl + 1) * QT, :]
    return out


# revision 25
# speedup vs baseline: 1.0330x; 1.0330x over previous
"""Causal multi-head self-attention with RoPE on 8 Trainium2 NeuronCores.

Sharding: batch (4) x query-half (2) -> 8 cores, no collectives.
Each core computes full K/V for its batch; query rows split between the two
cores of a batch in a causally-balanced schedule: 4 slots of 256 query rows,
slot sl covering C=4(sl+1) key blocks of 128 (j=0 tiles [1,3,5,7], j=1 tiles
[0,2,4,6]; masked tails make the shared SPMD program serve both halves).

Everything flows in fp16 (PSUM accumulation in f32) in transposed
[feature, seq] layout:
  K^T/Q^T = W^T.T @ X^T            per 128-row head pair
  RoPE    = cos*x + sin*(P@x)      (P = pair-rotation matrix)
  S^T     = Krot^T.T @ Qpad        per-head scores via a full 128-deep
                                   contraction against zero-padded Q (the
                                   other head's partition rows are zero), so
                                   no partition-offset matmuls are needed.
                                   2 heads x 2 key blocks pack into one
                                   2-bank PSUM tile -> single [128,1024] Exp.
  A       = exp-stationary @ [V|1]   ([q, 65] outputs, 65-row matmuls;
                                      region-major accumulation: interleaved
                                      start/stop groups in one PSUM bank drop
                                      contributions on HW)
  y       = (A/denominator).T via PE transpose, then @ Wo^T

Engine split: PE matmuls; Act = Exp + PSUM->SBUF K copies; DVE = rope
multiplies (PSUM direct), masks, spills, normalize; Pool (no PSUM access) =
SBUF-only memsets/scales.  Attention is emitted as 2-key-block pair units
software-pipelined into the projection stream so every engine stays busy.
"""

import os
import sys
import math

if "/opt/trn_rl_repo" not in sys.path:
    sys.path.append("/opt/trn_rl_repo")

import numpy as np

import concourse.bass as bass
import concourse.tile as tile
from concourse import bacc, mybir
from concourse.bass_utils import run_bass_kernel_spmd

B = 4
S = 2048
D = 1024
H = 16
DK = 64
THETA = 10000.0

NEP = H // 2          # head pairs (128-partition groups)
QT = 256              # query tile width
KB = 128              # key block
NSLOT = 4
TILES = [[1, 3, 5, 7], [0, 2, 4, 6]]  # slot -> 256-row q-tile, per half
VW = DK + 1           # V columns per head incl. trailing ones column

F32 = mybir.dt.float32
F16 = mybir.dt.float16

_cache = {}


def _build_program():
    if "nc" in _cache:
        return _cache["nc"]

    nc = bacc.Bacc("TRN2")

    xt_d = nc.dram_tensor("xt", [D, S], F16, kind="ExternalInput")
    xq_d = nc.dram_tensor("xq", [D, NSLOT * QT], F16, kind="ExternalInput")
    wkt_d = nc.dram_tensor("wkt", [D, D], F16, kind="ExternalInput")
    wqt_d = nc.dram_tensor("wqt", [D, D], F16, kind="ExternalInput")
    wvt_d = nc.dram_tensor("wvt", [D, D], F16, kind="ExternalInput")
    wot_d = nc.dram_tensor("wot", [D, D], F16, kind="ExternalInput")
    cosk_d = nc.dram_tensor("cosk", [128, S], F16, kind="ExternalInput")
    sink_d = nc.dram_tensor("sink", [128, S], F16, kind="ExternalInput")
    cosq_d = nc.dram_tensor("cosq", [128, NSLOT * QT], F16, kind="ExternalInput")
    sinq_d = nc.dram_tensor("sinq", [128, NSLOT * QT], F16, kind="ExternalInput")
    mask_d = nc.dram_tensor("mask", [128, NSLOT, 4, QT], F16, kind="ExternalInput")
    permt_d = nc.dram_tensor("permt", [128, 128], F16, kind="ExternalInput")
    ident_d = nc.dram_tensor("ident", [128, 128], F16, kind="ExternalInput")
    y_d = nc.dram_tensor("y", [NSLOT * QT, D], F32, kind="ExternalOutput")

    xt_t = xt_d.rearrange("(n p) s -> p n s", p=128)
    xq_t = xq_d.rearrange("(n p) s -> p n s", p=128)
    wkt_t = wkt_d.rearrange("(n p) e -> p n e", p=128)
    wqt_t = wqt_d.rearrange("(n p) e -> p n e", p=128)
    wvt_t = wvt_d.rearrange("(n p) e -> p n e", p=128)
    wot_t = wot_d.rearrange("(n p) e -> p n e", p=128)

    from contextlib import ExitStack

    with tile.TileContext(nc) as tc:
        with ExitStack() as stack:
            pool = lambda *a, **k: stack.enter_context(tc.tile_pool(*a, **k))
            cpool = pool(name="const", bufs=1)
            wkp = pool(name="wk", bufs=1)
            wqp = pool(name="wq", bufs=1)
            wvp = pool(name="wv", bufs=1)
            wop = pool(name="wo", bufs=1)
            ckp = pool(name="ck", bufs=1)
            cqp = pool(name="cq", bufs=1)
            kvp = pool(name="kv", bufs=1)
            qp = pool(name="qr", bufs=1)
            mkp = pool(name="mk", bufs=1)
            xsp = pool(name="xs", bufs=3)
            xqp = pool(name="xq", bufs=2)
            stg = pool(name="stg", bufs=2)
            exp_p = pool(name="ex", bufs=2)
            accsp = pool(name="acs", bufs=1)
            nrmp = pool(name="nrm", bufs=3)
            atp = pool(name="at", bufs=1)
            outs = pool(name="ot", bufs=1)
            pjp = pool(name="pj", bufs=2, space="PSUM")
            pbp = pool(name="pb", bufs=2, space="PSUM")
            pap = pool(name="pa", bufs=2, space="PSUM")

            # ------------- persistent tiles + preload DMAs (in need order) ---
            xs_t = {}   # (st, xh) -> [128, 8, 256]

            def load_xs(st, xh):
                t = xsp.tile([128, 8, QT], F16, tag="xs", name=f"xs{st}_{xh}")
                xs_t[(st, xh)] = t
                nc.sync.dma_start(
                    t[:], xt_t[:, :, st * 512 + xh * QT:st * 512 + (xh + 1) * QT])

            load_xs(0, 0)          # SP
            t = xsp.tile([128, 8, QT], F16, tag="xs", name="xs0_1")
            xs_t[(0, 1)] = t
            nc.scalar.dma_start(
                t[:], xt_t[:, :, QT:2 * QT])

            wk = [wkp.tile([128, 8, 128], F16, tag=f"wk{ep}", name=f"wk{ep}")
                  for ep in range(NEP)]
            permt = cpool.tile([128, 128], F16)
            wvtTt = [wvp.tile([128, 8, 512], F16, name=f"wvtT{i}")
                     for i in range(2)]
            wqTt = [wqp.tile([128, 8, 512], F16, name=f"wqT{i}")
                    for i in range(2)]
            xqs_t = {}
            xqs_t[0] = xqp.tile([128, 8, QT], F16, tag="xq", name="xq0")
            cosq = cqp.tile([128, NSLOT * QT], F16)
            sinq = cqp.tile([128, NSLOT * QT], F16)
            cosk = ckp.tile([128, S], F16)
            sink = ckp.tile([128, S], F16)
            masks = mkp.tile([128, NSLOT, 4, QT], F16)
            ident = cpool.tile([128, 128], F16)

            # strict first-use priority order, alternating the two HWDGE
            # queues (the DMA device serializes transfers globally).
            prio = [
                (wk[0][:], wkt_t[:, :, 0:128]),
                (wk[1][:], wkt_t[:, :, 128:256]),
                (wk[2][:], wkt_t[:, :, 256:384]),
                (wk[3][:], wkt_t[:, :, 384:512]),
                (permt[:], permt_d[:]),
                (wk[4][:], wkt_t[:, :, 512:640]),
                (wk[5][:], wkt_t[:, :, 640:768]),
                (wk[6][:], wkt_t[:, :, 768:896]),
                (wk[7][:], wkt_t[:, :, 896:1024]),
                (wvtTt[0][:], wvt_t[:, :, 0:512]),
                (wqTt[0][:], wqt_t[:, :, 0:512]),
                (xqs_t[0][:], xq_t[:, :, 0:QT]),
                (cosq[:], cosq_d[:]),
                (sinq[:], sinq_d[:]),
                (cosk[:], cosk_d[:]),
                (sink[:], sink_d[:]),
                (wvtTt[1][:], wvt_t[:, :, 512:D]),
                (wqTt[1][:], wqt_t[:, :, 512:D]),
                (masks[:], mask_d[:]),
                (ident[:], ident_d[:]),
            ]
            for i, (dst, srcp) in enumerate(prio):
                (nc.sync if i % 2 == 0 else nc.scalar).dma_start(dst, srcp)

            def wqT(d, lo, hi):
                return wqTt[lo // 512][:, d, lo % 512:(lo % 512) + (hi - lo)]

            load_xs(1, 0)
            t2 = xsp.tile([128, 8, QT], F16, tag="xs", name="xs1_1")
            xs_t[(1, 1)] = t2
            nc.scalar.dma_start(t2[:], xt_t[:, :, 512 + QT:512 + 2 * QT])

            # zero-padded qrot: [128, slot, head, q]; head h lives in
            # partition rows h*64..h*64+64, the other rows stay zero.
            qrot = [qp.tile([128, NSLOT, 2, QT], F16, tag=f"qr{ep}",
                            name=f"qr{ep}") for ep in range(NEP)]
            for ep in range(NEP):
                nc.gpsimd.memset(qrot[ep][:], 0.0)
            wotT = wop.tile([128, 8, D], F16, name="wotT")
            nc.gpsimd.dma_start(wotT[:, :, 0:512], wot_t[:, :, 0:512])
            nc.gpsimd.dma_start(wotT[:, :, 512:D], wot_t[:, :, 512:D])

            krot_t = {}   # (st, ep) -> [128, 512]
            vt_t = {}     # (st, half) -> [128, 16, VW]
            acc_s = {}    # (sl, ep) -> [128, 4, VW] f16 spill accumulator
            accp_of = {}  # (sl, ep) -> live PSUM accumulator
            aq_of = {}    # (sl, ep) -> normalized [128, 2, 2, 64] f16
            aT = {}       # ep -> [128, 2, 128] f16
            exctr = [0]   # alternates exp tile tags for deeper pipelining
            e_of = {}     # (sl, ep) -> exp tile of the chunk's first pair

            for sl in range(1, NSLOT):
                for ep in range(NEP):
                    acc_s[(sl, ep)] = accsp.tile(
                        [128, 4, VW], F16, tag=f"as{sl}_{ep}", name=f"as{sl}_{ep}")

            # ---------------- emission units ----------------
            def dma_xs(st, xh):
                def go():
                    load_xs(st, xh)
                return go

            def dma_xq(sl):
                def go():
                    t = xqp.tile([128, 8, QT], F16, tag="xq", name=f"xq{sl}")
                    xqs_t[sl] = t
                    nc.sync.dma_start(t[:], xq_t[:, :, sl * QT:(sl + 1) * QT])
                return go

            def u_K(st, ep):
                def go():
                    pk = pjp.tile([128, 512], F32, tag="pj", name="pk")
                    for xh in range(2):
                        for d in range(8):
                            nc.tensor.matmul(
                                pk[:, xh * QT:(xh + 1) * QT],
                                wk[ep][:, d, :],
                                xs_t[(st, xh)][:, d, :],
                                start=(d == 0), stop=(d == 7))
                    kraw = stg.tile([128, 512], F16, tag="kraw", name="kraw",
                                    bufs=1)
                    nc.scalar.activation(kraw[:], pk[:],
                                         mybir.ActivationFunctionType.Copy)
                    csl = slice(st * 512, (st + 1) * 512)
                    t_c = stg.tile([128, 512], F16, tag="tc", name="tc",
                                   bufs=1)
                    nc.vector.tensor_mul(t_c[:], kraw[:], cosk[:, csl])
                    pp = pjp.tile([128, 512], F32, tag="pj", name="pp")
                    nc.tensor.matmul(pp[:], permt[:], kraw[:],
                                     start=True, stop=True)
                    t_s = stg.tile([128, 512], F16, tag="ts", name="ts",
                                   bufs=1)
                    nc.vector.tensor_mul(t_s[:], pp[:], sink[:, csl])
                    kr = kvp.tile([128, 512], F16, tag=f"kr{ep}",
                                  name=f"kr{ep}_{st}", bufs=2)
                    krot_t[(st, ep)] = kr
                    nc.vector.tensor_add(kr[:], t_c[:], t_s[:])
                return go

            def u_Q(sl, ep):
                def go():
                    xq = xqs_t[sl]
                    pq = pjp.tile([128, 512], F32, tag="pj", name="pq")
                    for d in range(8):
                        nc.tensor.matmul(pq[:, 0:QT],
                                         wqT(d, ep * 128, (ep + 1) * 128),
                                         xq[:, d, :],
                                         start=(d == 0), stop=(d == 7))
                    qraw = stg.tile([128, QT], F16, tag="qraw", name="qraw")
                    nc.vector.tensor_copy(qraw[:], pq[:, 0:QT])
                    csl = slice(sl * QT, (sl + 1) * QT)
                    t_c = stg.tile([128, QT], F16, tag="tcq", name="tcq",
                                   bufs=1)
                    nc.vector.tensor_mul(t_c[:], qraw[:], cosq[:, csl])
                    pp = pjp.tile([128, 512], F32, tag="pj", name="ppq")
                    nc.tensor.matmul(pp[:, 0:QT], permt[:], qraw[:],
                                     start=True, stop=True)
                    t_s = stg.tile([128, QT], F16, tag="tsq", name="tsq",
                                   bufs=1)
                    nc.vector.tensor_mul(t_s[:], pp[:, 0:QT], sinq[:, csl])
                    for h in range(2):
                        nc.vector.tensor_add(
                            qrot[ep][h * 64:(h + 1) * 64, sl, h, :],
                            t_c[h * 64:(h + 1) * 64, :],
                            t_s[h * 64:(h + 1) * 64, :])
                return go

            def u_V(st, half):
                def go():
                    xh, off = half // 2, (half % 2) * KB
                    xs = xs_t[(st, xh)]
                    vtile = kvp.tile([128, H, VW], F16, tag=f"vt{half}",
                                     name=f"vt{half}_{st}", bufs=2)
                    vt_t[(st, half)] = vtile
                    nc.gpsimd.memset(vtile[:, :, DK], 1.0)
                    for et in range(2):
                        pv = pjp.tile([128, 512], F32, tag="pj", name="pv")
                        for d in range(8):
                            nc.tensor.matmul(
                                pv[:], xs[:, d, off:off + KB],
                                wvtTt[et][:, d, :],
                                start=(d == 0), stop=(d == 7))
                        if et == 0:
                            nc.scalar.activation(
                                vtile[:, et * 8:(et + 1) * 8, 0:DK],
                                pv[:].rearrange("p (h w) -> p h w", w=DK),
                                mybir.ActivationFunctionType.Copy)
                        else:
                            nc.vector.tensor_copy(
                                vtile[:, et * 8:(et + 1) * 8, 0:DK],
                                pv[:].rearrange("p (h w) -> p h w", w=DK))
                return go

            def u_pair(sl, cs, ep, pr, sched):
                C = 4 * (sl + 1)

                def go():
                    if pr == 0:
                        accp = pap.tile([128, 4, VW], F32, tag="acc",
                                        name="acc")
                        accp_of[(sl, ep)] = accp
                    else:
                        accp = accp_of[(sl, ep)]
                    psc = pbp.tile([128, 4, QT], F32, tag="pb", name="psc")
                    for i in range(2):
                        kb = 4 * cs + 2 * pr + i
                        kloc = kb - 4 * cs
                        for h in range(2):
                            nc.tensor.matmul(
                                psc[:, i * 2 + h, :],
                                krot_t[(cs, ep)][:, kloc * KB:(kloc + 1) * KB],
                                qrot[ep][:, sl, h, :],
                                start=True, stop=True)
                    tagx = "ex" if exctr[0] % 2 == 0 else "exm"
                    exctr[0] += 1
                    e = exp_p.tile([128, 4, QT], F16, tag=tagx, name="ex")
                    nc.scalar.activation(
                        e[:], psc[:], mybir.ActivationFunctionType.Exp,
                        scale=1.0 / math.sqrt(DK))
                    if 4 * cs + 2 * pr + 1 >= C - 4:
                        for i in range(2):
                            kb = 4 * cs + 2 * pr + i
                            m = kb - (C - 4)
                            for h in range(2):
                                nc.vector.tensor_mul(
                                    e[:, i * 2 + h, :], e[:, i * 2 + h, :],
                                    masks[:, sl, m, :])
                    if pr == 0:
                        e_of[(sl, ep)] = e
                        return

                    e0 = e_of[(sl, ep)]
                    e1 = e

                    def av_and_tail():
                        # Region-major accumulation: interleaved start/stop
                        # groups within one PSUM bank drop contributions on
                        # real HW, so each [q,65] region accumulates its 4
                        # key blocks consecutively.
                        for qh in range(2):
                            for h in range(2):
                                for kk in range(4):
                                    esrc = e0 if kk < 2 else e1
                                    nc.tensor.matmul(
                                        accp[:, qh * 2 + h, :],
                                        esrc[:, (kk % 2) * 2 + h,
                                             qh * 128:(qh + 1) * 128],
                                        vt_t[(cs, kk)][:, 2 * ep + h, :],
                                        start=(kk == 0), stop=(kk == 3))
                        if sl > 0 and cs < sl:
                            if cs == 0:
                                nc.vector.tensor_copy(
                                    acc_s[(sl, ep)][:], accp[:])
                            else:
                                nc.vector.tensor_add(
                                    acc_s[(sl, ep)][:], acc_s[(sl, ep)][:],
                                    accp[:])
                    sched.pend.append([0, av_and_tail])
                return go

            def u_fin_a(sl, ep):
                def go():
                    accp = accp_of[(sl, ep)]
                    if sl == 0:
                        src = accp
                    else:
                        src = nrmp.tile([128, 4, VW], F32, tag="accf",
                                        name="accf")
                        nc.vector.tensor_add(src[:], accp[:],
                                             acc_s[(sl, ep)][:])
                    rcp = nrmp.tile([128, 4], F32, tag="rcp", name="rcp")
                    nc.vector.reciprocal(rcp[:], src[:, :, DK])
                    aq = nrmp.tile([128, 2, 2, DK], F16, tag="aq", name="aq")
                    aq_of[(sl, ep)] = aq
                    for qh in range(2):
                        for h in range(2):
                            i = qh * 2 + h
                            eng = nc.vector if sl == 0 else nc.gpsimd
                            eng.tensor_scalar_mul(
                                aq[:, qh, h, :], src[:, i, 0:DK],
                                rcp[:, i:i + 1])
                return go

            def u_fin_b(sl, ep):
                def go():
                    aq = aq_of[(sl, ep)]
                    ptt = pap.tile([128, 2, 128], F16, tag="acc", name="ptt")
                    for qh in range(2):
                        nc.tensor.transpose(
                            ptt[:, qh, :],
                            aq[:, qh, :, :].rearrange("p a b -> p (a b)"),
                            ident[:])
                    t = atp.tile([128, 2, 128], F16, tag=f"aT{ep}",
                                 name=f"aT{ep}")
                    aT[ep] = t
                    nc.vector.tensor_copy(t[:], ptt[:])
                return go

            def u_out(sl, qs, et):
                def go():
                    po = pjp.tile([128, 512], F32, tag="pj", name="po")
                    for d in range(8):
                        nc.tensor.matmul(
                            po[:], aT[d][:, qs, :],
                            wotT[:, d, et * 512:(et + 1) * 512],
                            start=(d == 0), stop=(d == 7))
                    ot = outs.tile([128, 512], F32, tag=f"ot{et}", name="ot")
                    if sl == 3 and et == 0:
                        nc.scalar.activation(
                            ot[:], po[:], mybir.ActivationFunctionType.Copy)
                    else:
                        nc.vector.tensor_copy(ot[:], po[:])
                    dq = nc.sync if et == 0 else nc.scalar
                    dq.dma_start(
                        y_d[sl * QT + qs * 128: sl * QT + (qs + 1) * 128,
                            et * 512:(et + 1) * 512], ot[:])
                return go

            # ---------------- schedule ----------------
            class Sched:
                def __init__(self):
                    self.pend = []  # [age, fn]

                def boundary(self, final=False):
                    keep = []
                    for ent in self.pend:
                        if ent[0] >= 2 or final:
                            ent[1]()
                        else:
                            ent[0] += 1
                            keep.append(ent)
                    self.pend = keep

            sched = Sched()

            # Preamble compute: K(0) only; everything else is paced.
            for ep in range(NEP):
                sched.boundary()
                u_K(0, ep)()

            # Projection stream; per-unit thunks, emission-paced.
            P = []
            P += [u_V(0, half) for half in range(4)]          # 0-3
            P += [u_Q(0, ep) for ep in range(NEP)]            # 4-11
            P += [dma_xq(1)]                                  # 12
            P += [u_Q(1, ep) for ep in range(NEP)]            # 13-20
            P += [dma_xq(2), dma_xs(2, 0), dma_xs(2, 1)]      # 21-23
            P += [u_Q(2, ep) for ep in range(NEP)]            # 24-31
            P += [dma_xq(3)]                                  # 32
            P += [u_Q(3, ep) for ep in range(NEP)]            # 33-40
            P += [u_V(1, half) for half in range(4)]          # 41-44
            P += [u_K(1, ep) for ep in range(NEP)]            # 45-52
            P += [dma_xs(3, 0), dma_xs(3, 1)]                 # 53-54
            P += [u_V(2, half) for half in range(4)]          # 55-58
            P += [u_K(2, ep) for ep in range(NEP)]            # 59-66
            P += [u_V(3, half) for half in range(4)]          # 67-70
            P += [u_K(3, ep) for ep in range(NEP)]            # 71-78

            qpos = [5, 14, 25, 34]   # P pos after which Q(sl, ep) is emitted

            def dep_chunk(sl, cs, ep):
                d = qpos[sl] + ep
                if cs == 1:
                    d = max(d, 46 + ep)
                elif cs == 2:
                    d = max(d, 60 + ep)
                elif cs == 3:
                    d = max(d, 72 + ep)
                return d

            def finale_seq(sl):
                seq = []
                for ep in range(NEP):
                    d = dep_chunk(sl, sl, ep)
                    seq.append((d, u_pair(sl, sl, ep, 0, sched)))
                    seq.append((d, u_pair(sl, sl, ep, 1, sched)))
                    if ep >= 1:
                        seq.append((d, u_fin_a(sl, ep - 1)))
                    if ep >= 2:
                        seq.append((d, u_fin_b(sl, ep - 2)))
                dl = dep_chunk(sl, sl, NEP - 1)
                seq.append((dl, u_fin_a(sl, NEP - 1)))
                seq.append((dl, u_fin_b(sl, NEP - 2)))
                seq.append((dl, u_fin_b(sl, NEP - 1)))
                for qs in range(2):
                    for et in range(2):
                        seq.append((dl, u_out(sl, qs, et)))
                return seq

            def merge(a, b):
                """Proportionally interleave two (dep, unit) lists."""
                out = []
                ia = ib = 0
                while ia < len(a) or ib < len(b):
                    if ib >= len(b):
                        out.append(a[ia]); ia += 1
                    elif ia >= len(a):
                        out.append(b[ib]); ib += 1
                    elif ia * len(b) <= ib * len(a):
                        out.append(a[ia]); ia += 1
                    else:
                        out.append(b[ib]); ib += 1
                return out

            R = [None] * 4
            R[0] = merge(finale_seq(0),
                         [(dep_chunk(sl, 0, ep), u_pair(sl, 0, ep, pr, sched))
                          for sl in range(1, 4) for ep in range(NEP)
                          for pr in range(2)])
            R[1] = merge(finale_seq(1),
                         [(dep_chunk(sl, 1, ep), u_pair(sl, 1, ep, pr, sched))
                          for sl in range(2, 4) for ep in range(NEP)
                          for pr in range(2)])
            R[2] = merge(finale_seq(2),
                         [(dep_chunk(3, 2, ep), u_pair(3, 2, ep, pr, sched))
                          for ep in range(NEP) for pr in range(2)])
            R[3] = finale_seq(3)
            A = R[0] + R[1] + R[2] + R[3]

            # piecewise-linear pacing targets (A index, P position): P must
            # reach each round's K/V block by that round's first unit.
            anchors = [(0, 0), (len(R[0]), 55),
                       (len(R[0]) + len(R[1]), 67), (len(A), len(P))]

            def pace(i):
                for (i0, p0), (i1, p1) in zip(anchors, anchors[1:]):
                    if i < i1:
                        return p0 + ((p1 - p0) * (i - i0)) // max(1, i1 - i0)
                return len(P)

            p = 0
            for i, (dep, unit) in enumerate(A):
                target = max(dep, pace(i))
                while p < target and p < len(P):
                    sched.boundary()
                    P[p]()
                    p += 1
                sched.boundary()
                unit()
            while p < len(P):
                sched.boundary()
                P[p]()
                p += 1
            sched.boundary(final=True)
            sched.boundary(final=True)

    nc.compile()
    nc.finalize()
    _cache["nc"] = nc
    return nc


def _rope_tables(pos):
    """cos/sin tables in [128, n] head-pair layout."""
    k = np.arange(DK // 2, dtype=np.float64)
    inv_freq = THETA ** (-2.0 * k / DK)
    ang = inv_freq[:, None] * np.asarray(pos, np.float64)[None, :]  # [32, n]
    cos64 = np.repeat(np.cos(ang), 2, axis=0)
    sin64 = np.repeat(np.sin(ang), 2, axis=0)
    return (np.ascontiguousarray(
                np.concatenate([cos64, cos64], axis=0)).astype(np.float16),
            np.ascontiguousarray(
                np.concatenate([sin64, sin64], axis=0)).astype(np.float16))


def _masks(j):
    """[128, NSLOT, 4, QT] f16 multiplicative causal masks for half j."""
    p = np.arange(KB)[:, None]
    f = np.arange(QT)[None, :]
    triA = (f >= p).astype(np.float32)
    triB = (f >= p + KB).astype(np.float32)
    ones = np.ones((KB, QT), np.float32)
    zeros = np.zeros((KB, QT), np.float32)
    per_slot = [ones, ones, triA, triB] if j == 0 else [triA, triB, zeros, zeros]
    m = np.stack([np.stack(per_slot, axis=0)] * NSLOT, axis=0)  # [slot, 4, p, f]
    return np.ascontiguousarray(m.transpose(2, 0, 1, 3)).astype(np.float16)


def _host_inputs(in_features, token_positions, Wq, Wk, Wv, Wo):
    X = np.asarray(in_features, dtype=np.float32)
    pos = np.asarray(token_positions)
    wqt = np.ascontiguousarray(np.asarray(Wq, np.float32).T).astype(np.float16)
    wkt = np.ascontiguousarray(np.asarray(Wk, np.float32).T).astype(np.float16)
    wvt = np.ascontiguousarray(np.asarray(Wv, np.float32).T).astype(np.float16)
    wot = np.ascontiguousarray(np.asarray(Wo, np.float32).T).astype(np.float16)
    cosk, sink = _rope_tables(pos)

    permt = np.zeros((128, 128), np.float16)
    for i in range(64):
        permt[2 * i + 1, 2 * i] = -1.0
        permt[2 * i, 2 * i + 1] = 1.0
    ident = np.eye(128, dtype=np.float16)

    in_maps = []
    for core in range(8):
        b, j = core // 2, core % 2
        rows = np.concatenate(
            [np.arange(t * QT, (t + 1) * QT) for t in TILES[j]])
        cosq, sinq = _rope_tables(pos[rows])
        in_maps.append({
            "xt": np.ascontiguousarray(X[b].T).astype(np.float16),
            "xq": np.ascontiguousarray(X[b][rows].T).astype(np.float16),
            "wkt": wkt, "wqt": wqt, "wvt": wvt, "wot": wot,
            "cosk": cosk, "sink": sink, "cosq": cosq, "sinq": sinq,
            "mask": _masks(j), "permt": permt, "ident": ident,
        })
    return in_maps


def kernel(in_features, token_positions, Wq, Wk, Wv, Wo):
    nc = _build_program()
    in_maps = _host_inputs(in_features, token_positions, Wq, Wk, Wv, Wo)

    trace = bool(int(os.environ.get("KERNEL_TRACE", "0")))
    res = run_bass_kernel_spmd(nc, in_maps, core_ids=list(range(8)), trace=trace)
    kernel.last_result = res

    out = np.empty((B, S, D), np.float32)
    for core in range(8):
        b, j = core // 2, core % 2
        y = res.results[core]["y"]
        for sl, t in enumerate(TILES[j]):
            out[b, t * QT:(t + 1) * QT, :] = y[sl * QT:(sl + 1) * QT, :]
    return out


# revision 30
# speedup vs baseline: 1.0339x; 1.0009x over previous
"""Causal multi-head self-attention with RoPE on 8 Trainium2 NeuronCores.

Sharding: batch (4) x query-half (2) -> 8 cores, no collectives.
Each core computes full K/V for its batch; query rows split between the two
cores of a batch in a causally-balanced schedule: 4 slots of 256 query rows,
slot sl covering C=4(sl+1) key blocks of 128 (j=0 tiles [1,3,5,7], j=1 tiles
[0,2,4,6]; masked tails make the shared SPMD program serve both halves).

Everything flows in fp16 (PSUM accumulation in f32) in transposed
[feature, seq] layout:
  K^T/Q^T = W^T.T @ X^T            per 128-row head pair
  RoPE    = cos*x + sin*(P@x)      (P = pair-rotation matrix)
  S^T     = Krot^T.T @ Qpad        per-head scores via a full 128-deep
                                   contraction against zero-padded Q (the
                                   other head's partition rows are zero), so
                                   no partition-offset matmuls are needed.
                                   2 heads x 2 key blocks pack into one
                                   2-bank PSUM tile -> single [128,1024] Exp.
  A       = exp-stationary @ [V|1]   ([q, 65] outputs, 65-row matmuls;
                                      region-major accumulation: interleaved
                                      start/stop groups in one PSUM bank drop
                                      contributions on HW)
  y       = (A/denominator).T via PE transpose, then @ Wo^T

Engine split: PE matmuls; Act = Exp + PSUM->SBUF K copies; DVE = rope
multiplies (PSUM direct), masks, spills, normalize; Pool (no PSUM access) =
SBUF-only memsets/scales.  Attention is emitted as 2-key-block pair units
software-pipelined into the projection stream so every engine stays busy.
"""

import os
import sys
import math

if "/opt/trn_rl_repo" not in sys.path:
    sys.path.append("/opt/trn_rl_repo")

import numpy as np

import concourse.bass as bass
import concourse.tile as tile
from concourse import bacc, mybir
from concourse.bass_utils import run_bass_kernel_spmd

B = 4
S = 2048
D = 1024
H = 16
DK = 64
THETA = 10000.0

NEP = H // 2          # head pairs (128-partition groups)
QT = 256              # query tile width
KB = 128              # key block
NSLOT = 4
TILES = [[1, 3, 5, 7], [0, 2, 4, 6]]  # slot -> 256-row q-tile, per half
VW = DK + 1           # V columns per head incl. trailing ones column

F32 = mybir.dt.float32
F16 = mybir.dt.float16

_cache = {}


def _build_program():
    if "nc" in _cache:
        return _cache["nc"]

    nc = bacc.Bacc("TRN2")

    xt_d = nc.dram_tensor("xt", [D, S], F16, kind="ExternalInput")
    xq_d = nc.dram_tensor("xq", [D, NSLOT * QT], F16, kind="ExternalInput")
    wkt_d = nc.dram_tensor("wkt", [D, D], F16, kind="ExternalInput")
    wqt_d = nc.dram_tensor("wqt", [D, D], F16, kind="ExternalInput")
    wvt_d = nc.dram_tensor("wvt", [D, D], F16, kind="ExternalInput")
    wot_d = nc.dram_tensor("wot", [D, D], F16, kind="ExternalInput")
    cosk_d = nc.dram_tensor("cosk", [128, S], F16, kind="ExternalInput")
    sink_d = nc.dram_tensor("sink", [128, S], F16, kind="ExternalInput")
    cosq_d = nc.dram_tensor("cosq", [128, NSLOT * QT], F16, kind="ExternalInput")
    sinq_d = nc.dram_tensor("sinq", [128, NSLOT * QT], F16, kind="ExternalInput")
    mask_d = nc.dram_tensor("mask", [128, NSLOT, 4, QT], F16, kind="ExternalInput")
    permt_d = nc.dram_tensor("permt", [128, 128], F16, kind="ExternalInput")
    ident_d = nc.dram_tensor("ident", [128, 128], F16, kind="ExternalInput")
    y_d = nc.dram_tensor("y", [NSLOT * QT, D], F32, kind="ExternalOutput")

    xt_t = xt_d.rearrange("(n p) s -> p n s", p=128)
    xq_t = xq_d.rearrange("(n p) s -> p n s", p=128)
    wkt_t = wkt_d.rearrange("(n p) e -> p n e", p=128)
    wqt_t = wqt_d.rearrange("(n p) e -> p n e", p=128)
    wvt_t = wvt_d.rearrange("(n p) e -> p n e", p=128)
    wot_t = wot_d.rearrange("(n p) e -> p n e", p=128)

    from contextlib import ExitStack

    with tile.TileContext(nc) as tc:
        with ExitStack() as stack:
            pool = lambda *a, **k: stack.enter_context(tc.tile_pool(*a, **k))
            cpool = pool(name="const", bufs=1)
            wkp = pool(name="wk", bufs=1)
            wqp = pool(name="wq", bufs=1)
            wvp = pool(name="wv", bufs=1)
            wop = pool(name="wo", bufs=1)
            ckp = pool(name="ck", bufs=1)
            cqp = pool(name="cq", bufs=1)
            kvp = pool(name="kv", bufs=1)
            qp = pool(name="qr", bufs=1)
            mkp = pool(name="mk", bufs=1)
            xsp = pool(name="xs", bufs=3)
            xqp = pool(name="xq", bufs=2)
            stg = pool(name="stg", bufs=2)
            exp_p = pool(name="ex", bufs=2)
            accsp = pool(name="acs", bufs=1)
            nrmp = pool(name="nrm", bufs=3)
            atp = pool(name="at", bufs=1)
            outs = pool(name="ot", bufs=1)
            pjp = pool(name="pj", bufs=2, space="PSUM")
            pbp = pool(name="pb", bufs=2, space="PSUM")
            pap = pool(name="pa", bufs=2, space="PSUM")

            # ------------- persistent tiles + preload DMAs (in need order) ---
            xs_t = {}   # (st, xh) -> [128, 8, 256]

            def load_xs(st, xh):
                t = xsp.tile([128, 8, QT], F16, tag="xs", name=f"xs{st}_{xh}")
                xs_t[(st, xh)] = t
                nc.sync.dma_start(
                    t[:], xt_t[:, :, st * 512 + xh * QT:st * 512 + (xh + 1) * QT])

            load_xs(0, 0)          # SP
            t = xsp.tile([128, 8, QT], F16, tag="xs", name="xs0_1")
            xs_t[(0, 1)] = t
            nc.scalar.dma_start(
                t[:], xt_t[:, :, QT:2 * QT])

            wk = [wkp.tile([128, 8, 128], F16, tag=f"wk{ep}", name=f"wk{ep}")
                  for ep in range(NEP)]
            permt = cpool.tile([128, 128], F16)
            wvtTt = [wvp.tile([128, 8, 512], F16, name=f"wvtT{i}")
                     for i in range(2)]
            wqTt = [wqp.tile([128, 8, 512], F16, name=f"wqT{i}")
                    for i in range(2)]
            xqs_t = {}
            xqs_t[0] = xqp.tile([128, 8, QT], F16, tag="xq", name="xq0")
            cosq = cqp.tile([128, NSLOT * QT], F16)
            sinq = cqp.tile([128, NSLOT * QT], F16)
            cosk = ckp.tile([128, S], F16)
            sink = ckp.tile([128, S], F16)
            masks = mkp.tile([128, NSLOT, 4, QT], F16)
            ident = cpool.tile([128, 128], F16)

            # strict first-use priority order, alternating the two HWDGE
            # queues (the DMA device serializes transfers globally).
            prio = [
                (wk[0][:], wkt_t[:, :, 0:128]),
                (wk[1][:], wkt_t[:, :, 128:256]),
                (wk[2][:], wkt_t[:, :, 256:384]),
                (wk[3][:], wkt_t[:, :, 384:512]),
                (permt[:], permt_d[:]),
                (wk[4][:], wkt_t[:, :, 512:640]),
                (wk[5][:], wkt_t[:, :, 640:768]),
                (wk[6][:], wkt_t[:, :, 768:896]),
                (wk[7][:], wkt_t[:, :, 896:1024]),
                (wvtTt[0][:], wvt_t[:, :, 0:512]),
                (wqTt[0][:], wqt_t[:, :, 0:512]),
                (xqs_t[0][:], xq_t[:, :, 0:QT]),
                (cosq[:], cosq_d[:]),
                (sinq[:], sinq_d[:]),
                (cosk[:], cosk_d[:]),
                (sink[:], sink_d[:]),
                (wvtTt[1][:], wvt_t[:, :, 512:D]),
                (wqTt[1][:], wqt_t[:, :, 512:D]),
                (masks[:], mask_d[:]),
                (ident[:], ident_d[:]),
            ]
            for i, (dst, srcp) in enumerate(prio):
                (nc.sync if i % 2 == 0 else nc.scalar).dma_start(dst, srcp)

            def wqT(d, lo, hi):
                return wqTt[lo // 512][:, d, lo % 512:(lo % 512) + (hi - lo)]

            load_xs(1, 0)
            t2 = xsp.tile([128, 8, QT], F16, tag="xs", name="xs1_1")
            xs_t[(1, 1)] = t2
            nc.scalar.dma_start(t2[:], xt_t[:, :, 512 + QT:512 + 2 * QT])

            # zero-padded qrot: [128, slot, head, q]; head h lives in
            # partition rows h*64..h*64+64, the other rows stay zero.
            qrot = [qp.tile([128, NSLOT, 2, QT], F16, tag=f"qr{ep}",
                            name=f"qr{ep}") for ep in range(NEP)]
            for ep in range(NEP):
                nc.gpsimd.memset(qrot[ep][:], 0.0)
            wotT = wop.tile([128, 8, D], F16, name="wotT")
            nc.gpsimd.dma_start(wotT[:, :, 0:512], wot_t[:, :, 0:512])
            nc.gpsimd.dma_start(wotT[:, :, 512:D], wot_t[:, :, 512:D])

            krot_t = {}   # (st, ep) -> [128, 512]
            vt_t = {}     # (st, half) -> [128, 16, VW]
            acc_s = {}    # (sl, ep) -> [128, 4, VW] f16 spill accumulator
            accp_of = {}  # (sl, ep) -> live PSUM accumulator
            aq_of = {}    # (sl, ep) -> normalized [128, 2, 2, 64] f16
            aT = {}       # ep -> [128, 2, 128] f16
            exctr = [0]   # alternates exp tile tags for deeper pipelining
            e_of = {}     # (sl, ep) -> exp tile of the chunk's first pair

            for sl in range(1, NSLOT):
                for ep in range(NEP):
                    acc_s[(sl, ep)] = accsp.tile(
                        [128, 4, VW], F16, tag=f"as{sl}_{ep}", name=f"as{sl}_{ep}")

            # ---------------- emission units ----------------
            def dma_xs(st, xh):
                def go():
                    load_xs(st, xh)
                return go

            def dma_xq(sl):
                def go():
                    t = xqp.tile([128, 8, QT], F16, tag="xq", name=f"xq{sl}")
                    xqs_t[sl] = t
                    nc.sync.dma_start(t[:], xq_t[:, :, sl * QT:(sl + 1) * QT])
                return go

            def u_K(st, ep):
                def go():
                    pk = pjp.tile([128, 512], F32, tag="pj", name="pk")
                    for xh in range(2):
                        for d in range(8):
                            nc.tensor.matmul(
                                pk[:, xh * QT:(xh + 1) * QT],
                                wk[ep][:, d, :],
                                xs_t[(st, xh)][:, d, :],
                                start=(d == 0), stop=(d == 7))
                    kraw = stg.tile([128, 512], F16, tag="kraw", name="kraw",
                                    bufs=1)
                    nc.scalar.activation(kraw[:], pk[:],
                                         mybir.ActivationFunctionType.Copy)
                    csl = slice(st * 512, (st + 1) * 512)
                    t_c = stg.tile([128, 512], F16, tag="tc", name="tc",
                                   bufs=1)
                    nc.vector.tensor_mul(t_c[:], kraw[:], cosk[:, csl])
                    pp = pjp.tile([128, 512], F32, tag="pj", name="pp")
                    nc.tensor.matmul(pp[:], permt[:], kraw[:],
                                     start=True, stop=True)
                    t_s = stg.tile([128, 512], F16, tag="ts", name="ts",
                                   bufs=1)
                    nc.vector.tensor_mul(t_s[:], pp[:], sink[:, csl])
                    kr = kvp.tile([128, 512], F16, tag=f"kr{ep}",
                                  name=f"kr{ep}_{st}", bufs=2)
                    krot_t[(st, ep)] = kr
                    nc.vector.tensor_add(kr[:], t_c[:], t_s[:])
                return go

            def u_Q(sl, ep):
                def go():
                    xq = xqs_t[sl]
                    pq = pjp.tile([128, 512], F32, tag="pj", name="pq")
                    for d in range(8):
                        nc.tensor.matmul(pq[:, 0:QT],
                                         wqT(d, ep * 128, (ep + 1) * 128),
                                         xq[:, d, :],
                                         start=(d == 0), stop=(d == 7))
                    qraw = stg.tile([128, QT], F16, tag="qraw", name="qraw")
                    nc.vector.tensor_copy(qraw[:], pq[:, 0:QT])
                    csl = slice(sl * QT, (sl + 1) * QT)
                    t_c = stg.tile([128, QT], F16, tag="tcq", name="tcq",
                                   bufs=1)
                    nc.vector.tensor_mul(t_c[:], qraw[:], cosq[:, csl])
                    pp = pjp.tile([128, 512], F32, tag="pj", name="ppq")
                    nc.tensor.matmul(pp[:, 0:QT], permt[:], qraw[:],
                                     start=True, stop=True)
                    t_s = stg.tile([128, QT], F16, tag="tsq", name="tsq",
                                   bufs=1)
                    nc.vector.tensor_mul(t_s[:], pp[:, 0:QT], sinq[:, csl])
                    for h in range(2):
                        nc.vector.tensor_add(
                            qrot[ep][h * 64:(h + 1) * 64, sl, h, :],
                            t_c[h * 64:(h + 1) * 64, :],
                            t_s[h * 64:(h + 1) * 64, :])
                return go

            def u_V(st, half):
                def go():
                    xh, off = half // 2, (half % 2) * KB
                    xs = xs_t[(st, xh)]
                    vtile = kvp.tile([128, H, VW], F16, tag=f"vt{half}",
                                     name=f"vt{half}_{st}", bufs=2)
                    vt_t[(st, half)] = vtile
                    nc.gpsimd.memset(vtile[:, :, DK], 1.0)
                    for et in range(2):
                        pv = pjp.tile([128, 512], F32, tag="pj", name="pv")
                        for d in range(8):
                            nc.tensor.matmul(
                                pv[:], xs[:, d, off:off + KB],
                                wvtTt[et][:, d, :],
                                start=(d == 0), stop=(d == 7))
                        if et == 0:
                            nc.scalar.activation(
                                vtile[:, et * 8:(et + 1) * 8, 0:DK],
                                pv[:].rearrange("p (h w) -> p h w", w=DK),
                                mybir.ActivationFunctionType.Copy)
                        else:
                            nc.vector.tensor_copy(
                                vtile[:, et * 8:(et + 1) * 8, 0:DK],
                                pv[:].rearrange("p (h w) -> p h w", w=DK))
                return go

            def u_pair(sl, cs, ep, pr, sched):
                C = 4 * (sl + 1)

                def go():
                    if pr == 0:
                        accp = pap.tile([128, 4, VW], F32, tag="acc",
                                        name="acc")
                        accp_of[(sl, ep)] = accp
                    else:
                        accp = accp_of[(sl, ep)]
                    psc = pbp.tile([128, 4, QT], F32, tag="pb", name="psc")
                    for i in range(2):
                        kb = 4 * cs + 2 * pr + i
                        kloc = kb - 4 * cs
                        for h in range(2):
                            nc.tensor.matmul(
                                psc[:, i * 2 + h, :],
                                krot_t[(cs, ep)][:, kloc * KB:(kloc + 1) * KB],
                                qrot[ep][:, sl, h, :],
                                start=True, stop=True)
                    tagx = "ex" if exctr[0] % 2 == 0 else "exm"
                    exctr[0] += 1
                    e = exp_p.tile([128, 4, QT], F16, tag=tagx, name="ex")
                    nc.scalar.activation(
                        e[:], psc[:], mybir.ActivationFunctionType.Exp,
                        scale=1.0 / math.sqrt(DK))
                    if 4 * cs + 2 * pr + 1 >= C - 4:
                        for i in range(2):
                            kb = 4 * cs + 2 * pr + i
                            m = kb - (C - 4)
                            for h in range(2):
                                nc.vector.tensor_mul(
                                    e[:, i * 2 + h, :], e[:, i * 2 + h, :],
                                    masks[:, sl, m, :])
                    if pr == 0:
                        e_of[(sl, ep)] = e
                        return

                    e0 = e_of[(sl, ep)]
                    e1 = e

                    def av_and_tail():
                        # Region-major accumulation: interleaved start/stop
                        # groups within one PSUM bank drop contributions on
                        # real HW, so each [q,65] region accumulates its 4
                        # key blocks consecutively.
                        for qh in range(2):
                            for h in range(2):
                                for kk in range(4):
                                    esrc = e0 if kk < 2 else e1
                                    nc.tensor.matmul(
                                        accp[:, qh * 2 + h, :],
                                        esrc[:, (kk % 2) * 2 + h,
                                             qh * 128:(qh + 1) * 128],
                                        vt_t[(cs, kk)][:, 2 * ep + h, :],
                                        start=(kk == 0), stop=(kk == 3))
                        if sl > 0 and cs < sl:
                            if cs == 0:
                                nc.vector.tensor_copy(
                                    acc_s[(sl, ep)][:], accp[:])
                            else:
                                nc.vector.tensor_add(
                                    acc_s[(sl, ep)][:], acc_s[(sl, ep)][:],
                                    accp[:])
                    sched.pend.append([0, av_and_tail])
                return go

            def u_fin_a(sl, ep):
                def go():
                    accp = accp_of[(sl, ep)]
                    if sl == 0:
                        src = accp
                    else:
                        src = nrmp.tile([128, 4, VW], F32, tag="accf",
                                        name="accf")
                        nc.vector.tensor_add(src[:], accp[:],
                                             acc_s[(sl, ep)][:])
                    rcp = nrmp.tile([128, 4], F32, tag="rcp", name="rcp")
                    nc.vector.reciprocal(rcp[:], src[:, :, DK])
                    aq = nrmp.tile([128, 2, 2, DK], F16, tag="aq", name="aq")
                    aq_of[(sl, ep)] = aq
                    for qh in range(2):
                        for h in range(2):
                            i = qh * 2 + h
                            eng = nc.vector if sl == 0 else nc.gpsimd
                            eng.tensor_scalar_mul(
                                aq[:, qh, h, :], src[:, i, 0:DK],
                                rcp[:, i:i + 1])
                return go

            def u_fin_b(sl, ep):
                def go():
                    aq = aq_of[(sl, ep)]
                    ptt = pap.tile([128, 2, 128], F16, tag="acc", name="ptt")
                    for qh in range(2):
                        nc.tensor.transpose(
                            ptt[:, qh, :],
                            aq[:, qh, :, :].rearrange("p a b -> p (a b)"),
                            ident[:])
                    t = atp.tile([128, 2, 128], F16, tag=f"aT{ep}",
                                 name=f"aT{ep}")
                    aT[ep] = t
                    nc.vector.tensor_copy(t[:], ptt[:])
                return go

            def u_out(sl, qs, et):
                def go():
                    po = pjp.tile([128, 512], F32, tag="pj", name="po")
                    for d in range(8):
                        nc.tensor.matmul(
                            po[:], aT[d][:, qs, :],
                            wotT[:, d, et * 512:(et + 1) * 512],
                            start=(d == 0), stop=(d == 7))
                    ot = outs.tile([128, 512], F32, tag=f"ot{et}", name="ot")
                    if sl == 3 and et == 0:
                        nc.scalar.activation(
                            ot[:], po[:], mybir.ActivationFunctionType.Copy)
                    else:
                        nc.vector.tensor_copy(ot[:], po[:])
                    dq = nc.sync if et == 0 else nc.scalar
                    dq.dma_start(
                        y_d[sl * QT + qs * 128: sl * QT + (qs + 1) * 128,
                            et * 512:(et + 1) * 512], ot[:])
                return go

            # ---------------- schedule ----------------
            class Sched:
                def __init__(self):
                    self.pend = []  # [age, fn]

                def boundary(self, final=False):
                    keep = []
                    for ent in self.pend:
                        if ent[0] >= 2 or final:
                            ent[1]()
                        else:
                            ent[0] += 1
                            keep.append(ent)
                    self.pend = keep

            sched = Sched()

            # Preamble compute: K(0,0..1) only; the rest is paced so PE
            # fills the startup DMA window instead of idling.
            for ep in range(2):
                sched.boundary()
                u_K(0, ep)()

            # Projection stream; per-unit thunks, emission-paced.
            P = []
            P += [u_K(0, ep) for ep in range(2, NEP)]         # 0-5
            P += [u_V(0, half) for half in range(4)]          # 6-9
            P += [u_Q(0, ep) for ep in range(NEP)]            # 10-17
            P += [dma_xq(1)]                                  # 18
            P += [u_Q(1, ep) for ep in range(NEP)]            # 19-26
            P += [dma_xq(2), dma_xs(2, 0), dma_xs(2, 1)]      # 27-29
            P += [u_Q(2, ep) for ep in range(NEP)]            # 30-37
            P += [dma_xq(3)]                                  # 38
            P += [u_Q(3, ep) for ep in range(NEP)]            # 39-46
            P += [u_V(1, half) for half in range(4)]          # 47-50
            P += [u_K(1, ep) for ep in range(NEP)]            # 51-58
            P += [dma_xs(3, 0), dma_xs(3, 1)]                 # 59-60
            P += [u_V(2, half) for half in range(4)]          # 61-64
            P += [u_K(2, ep) for ep in range(NEP)]            # 65-72
            P += [u_V(3, half) for half in range(4)]          # 73-76
            P += [u_K(3, ep) for ep in range(NEP)]            # 77-84

            qpos = [11, 20, 31, 40]  # P pos after which Q(sl, ep) is emitted

            def dep_chunk(sl, cs, ep):
                d = qpos[sl] + ep
                if cs == 1:
                    d = max(d, 52 + ep)
                elif cs == 2:
                    d = max(d, 66 + ep)
                elif cs == 3:
                    d = max(d, 78 + ep)
                return d

            def finale_seq(sl):
                seq = []
                for ep in range(NEP):
                    d = dep_chunk(sl, sl, ep)
                    seq.append((d, u_pair(sl, sl, ep, 0, sched)))
                    seq.append((d, u_pair(sl, sl, ep, 1, sched)))
                    if ep >= 1:
                        seq.append((d, u_fin_a(sl, ep - 1)))
                    if ep >= 2:
                        seq.append((d, u_fin_b(sl, ep - 2)))
                dl = dep_chunk(sl, sl, NEP - 1)
                seq.append((dl, u_fin_a(sl, NEP - 1)))
                seq.append((dl, u_fin_b(sl, NEP - 2)))
                seq.append((dl, u_fin_b(sl, NEP - 1)))
                for qs in range(2):
                    for et in range(2):
                        seq.append((dl, u_out(sl, qs, et)))
                return seq

            def merge(a, b):
                """Proportionally interleave two (dep, unit) lists."""
                out = []
                ia = ib = 0
                while ia < len(a) or ib < len(b):
                    if ib >= len(b):
                        out.append(a[ia]); ia += 1
                    elif ia >= len(a):
                        out.append(b[ib]); ib += 1
                    elif ia * len(b) <= ib * len(a):
                        out.append(a[ia]); ia += 1
                    else:
                        out.append(b[ib]); ib += 1
                return out

            R = [None] * 4
            R[0] = merge(finale_seq(0),
                         [(dep_chunk(sl, 0, ep), u_pair(sl, 0, ep, pr, sched))
                          for sl in range(1, 4) for ep in range(NEP)
                          for pr in range(2)])
            R[1] = merge(finale_seq(1),
                         [(dep_chunk(sl, 1, ep), u_pair(sl, 1, ep, pr, sched))
                          for sl in range(2, 4) for ep in range(NEP)
                          for pr in range(2)])
            R[2] = merge(finale_seq(2),
                         [(dep_chunk(3, 2, ep), u_pair(3, 2, ep, pr, sched))
                          for ep in range(NEP) for pr in range(2)])
            R[3] = finale_seq(3)
            A = R[0] + R[1] + R[2] + R[3]

            # piecewise-linear pacing targets (A index, P position): P must
            # reach each round's K/V block by that round's first unit.
            anchors = [(0, 0), (len(R[0]), 61),
                       (len(R[0]) + len(R[1]), 73), (len(A), len(P))]

            def pace(i):
                for (i0, p0), (i1, p1) in zip(anchors, anchors[1:]):
                    if i < i1:
                        return p0 + ((p1 - p0) * (i - i0)) // max(1, i1 - i0)
                return len(P)

            p = 0
            for i, (dep, unit) in enumerate(A):
                target = max(dep, pace(i))
                while p < target and p < len(P):
                    sched.boundary()
                    P[p]()
                    p += 1
                sched.boundary()
                unit()
            while p < len(P):
                sched.boundary()
                P[p]()
                p += 1
            sched.boundary(final=True)
            sched.boundary(final=True)

    nc.compile()
    nc.finalize()
    _cache["nc"] = nc
    return nc


def _rope_tables(pos):
    """cos/sin tables in [128, n] head-pair layout."""
    k = np.arange(DK // 2, dtype=np.float64)
    inv_freq = THETA ** (-2.0 * k / DK)
    ang = inv_freq[:, None] * np.asarray(pos, np.float64)[None, :]  # [32, n]
    cos64 = np.repeat(np.cos(ang), 2, axis=0)
    sin64 = np.repeat(np.sin(ang), 2, axis=0)
    return (np.ascontiguousarray(
                np.concatenate([cos64, cos64], axis=0)).astype(np.float16),
            np.ascontiguousarray(
                np.concatenate([sin64, sin64], axis=0)).astype(np.float16))


def _masks(j):
    """[128, NSLOT, 4, QT] f16 multiplicative causal masks for half j."""
    p = np.arange(KB)[:, None]
    f = np.arange(QT)[None, :]
    triA = (f >= p).astype(np.float32)
    triB = (f >= p + KB).astype(np.float32)
    ones = np.ones((KB, QT), np.float32)
    zeros = np.zeros((KB, QT), np.float32)
    per_slot = [ones, ones, triA, triB] if j == 0 else [triA, triB, zeros, zeros]
    m = np.stack([np.stack(per_slot, axis=0)] * NSLOT, axis=0)  # [slot, 4, p, f]
    return np.ascontiguousarray(m.transpose(2, 0, 1, 3)).astype(np.float16)


def _host_inputs(in_features, token_positions, Wq, Wk, Wv, Wo):
    X = np.asarray(in_features, dtype=np.float32)
    pos = np.asarray(token_positions)
    wqt = np.ascontiguousarray(np.asarray(Wq, np.float32).T).astype(np.float16)
    wkt = np.ascontiguousarray(np.asarray(Wk, np.float32).T).astype(np.float16)
    wvt = np.ascontiguousarray(np.asarray(Wv, np.float32).T).astype(np.float16)
    wot = np.ascontiguousarray(np.asarray(Wo, np.float32).T).astype(np.float16)
    cosk, sink = _rope_tables(pos)

    permt = np.zeros((128, 128), np.float16)
    for i in range(64):
        permt[2 * i + 1, 2 * i] = -1.0
        permt[2 * i, 2 * i + 1] = 1.0
    ident = np.eye(128, dtype=np.float16)

    in_maps = []
    for core in range(8):
        b, j = core // 2, core % 2
        rows = np.concatenate(
            [np.arange(t * QT, (t + 1) * QT) for t in TILES[j]])
        cosq, sinq = _rope_tables(pos[rows])
        in_maps.append({
            "xt": np.ascontiguousarray(X[b].T).astype(np.float16),
            "xq": np.ascontiguousarray(X[b][rows].T).astype(np.float16),
            "wkt": wkt, "wqt": wqt, "wvt": wvt, "wot": wot,
            "cosk": cosk, "sink": sink, "cosq": cosq, "sinq": sinq,
            "mask": _masks(j), "permt": permt, "ident": ident,
        })
    return in_maps


def kernel(in_features, token_positions, Wq, Wk, Wv, Wo):
    nc = _build_program()
    in_maps = _host_inputs(in_features, token_positions, Wq, Wk, Wv, Wo)

    trace = bool(int(os.environ.get("KERNEL_TRACE", "0")))
    res = run_bass_kernel_spmd(nc, in_maps, core_ids=list(range(8)), trace=trace)
    kernel.last_result = res

    out = np.empty((B, S, D), np.float32)
    for core in range(8):
        b, j = core // 2, core % 2
        y = res.results[core]["y"]
        for sl, t in enumerate(TILES[j]):
            out[b, t * QT:(t + 1) * QT, :] = y[sl * QT:(sl + 1) * QT, :]
    return out
